# revision 10
# baseline (speedup 1.0000x reference)
"""Trainium2 Bass kernel for nn_Efficient8BitALU_AddSub (v2).

Contract: kernel(**inputs) takes FULL unsharded inputs (numpy), returns FULL
output [32, 2048, 128] float32.  Internally shards tokens across 8 NeuronCores
(pure data parallel), runs a Bass/Tile kernel per core, gathers.

v2 redesign vs baseline (63.6us):
  - DMA layout "(p n) f": each partition holds 64 consecutive tokens, so every
    HBM descriptor moves 4KB contiguous (was 512B rows) -> near line rate.
  - c-vector transpose moved off PE onto the DMA xbar (dma_start_transpose,
    SBUF->SBUF, doesn't touch HBM), killing 64 PE transposes + ACT copies.
  - h-matmul: 32 N=512 matmuls with masked A/B weight blocks per 32-row band
    (pos0 comps at band rows 0:16, pos1 at 16:32), W16 replicated at the four
    row-group offsets.  layer2 keeps lhsT=RH chunks (tokens end on partitions).
  - decode in bf16, window min-reduce on gpsimd, fewer/bigger DVE ops.
  - outputs DMA'd via gpsimd SWDGE ring so the SP ring only carries inputs +
    xbar transposes (no FIFO stalls).
"""

import sys

import numpy as np

sys.path.insert(0, "/opt/trn_rl_repo")

import ml_dtypes  # noqa: E402
import concourse.bacc as bacc  # noqa: E402
import concourse.bass as bass  # noqa: E402
import concourse.mybir as mybir  # noqa: E402
import concourse.tile as tile  # noqa: E402

dt = mybir.dt
Alu = mybir.AluOpType
Act = mybir.ActivationFunctionType

# ---- problem constants (hardcoded per contract) ----
B, S, D = 32, 2048, 128
NCORES = 8
TOK = B * S                   # 65536
TPC = TOK // NCORES           # 8192 tokens per core

WIN0 = 3                      # 4 contiguous 16-wide decode windows: 3..66
OUT_LO = 67                   # outputs 67..98 (lo 67:83, hi 83:99)
OPA, OPS = 124, 125
GE_RESULT = 63
ROUND_C = 12582912.0          # 1.5 * 2**23 : RNE round-to-integer magic

NT = TPC // 128               # 64 tiles; token t = p*NT + n
NST = 2                       # supertiles
G = NT // NST                 # 32 tiles per supertile
NCHUNK = 8                    # dma chunks of 8 tiles

REDUCE_ON_GPSIMD = False  # gpsimd tensor_reduce is partition-axis only


def build_nc():
    nc = bacc.Bacc("TRN2", target_bir_lowering=False, debug=False,
                   num_devices=NCORES)
    xd = nc.dram_tensor("xc", [TPC, D], dt.float32, kind="ExternalInput")
    w16d = nc.dram_tensor("cW16AB", [128, 256], dt.bfloat16, kind="ExternalInput")
    w2d = nc.dram_tensor("cW2", [128, 2], dt.float16, kind="ExternalInput")
    iotad = nc.dram_tensor("cIOTA", [128, 32], dt.bfloat16, kind="ExternalInput")
    k16d = nc.dram_tensor("cK16", [128, 64], dt.bfloat16, kind="ExternalInput")
    yd = nc.dram_tensor("yc", [TPC, D], dt.float32, kind="ExternalOutput")

    # token t = p*NT + n : per-partition-contiguous DRAM blocks
    xr = xd.ap().rearrange("(p n) f -> p n f", n=NT)
    yr = yd.ap().rearrange("(p n) f -> p n f", n=NT)

    tpc = NT // NCHUNK        # tiles per dma chunk = 8

    with tile.TileContext(nc) as tc:
        with (
            tc.tile_pool(name="const", bufs=1) as cpool,
            tc.tile_pool(name="big", bufs=1) as bpool,
            tc.tile_pool(name="hp", bufs=2, space="PSUM") as hp_pool,
            tc.tile_pool(name="rp", bufs=2, space="PSUM") as rp_pool,
        ):
            W16 = cpool.tile([128, 256], dt.bfloat16, tag="w16")
            W2 = cpool.tile([128, 2], dt.float16, tag="w2")
            IOTA = cpool.tile([128, 32], dt.bfloat16, tag="iota")
            K16 = cpool.tile([128, 64], dt.bfloat16, tag="k16")
            nc.gpsimd.dma_start(W16[:], w16d.ap())
            nc.gpsimd.dma_start(W2[:], w2d.ap())
            nc.gpsimd.dma_start(IOTA[:], iotad.ap())
            nc.gpsimd.dma_start(K16[:], k16d.ap())

            X = bpool.tile([128, NT * 128], dt.float32, tag="X")
            XR = X[:].rearrange("p (n f) -> p n f", f=128)

            HT = bpool.tile([128, NT * 64], dt.bfloat16, tag="HT")
            HR = HT[:].rearrange("p (n w) -> p n w", w=64)
            NIB = bpool.tile([128, NT * 4], dt.bfloat16, tag="NIB")
            NIB4 = NIB[:].rearrange("p (n w) -> p n w", w=4)
            NM = bpool.tile([128, NT * 4], dt.bfloat16, tag="NM")
            NM4 = NM[:].rearrange("p (n w) -> p n w", w=4)
            FL = bpool.tile([128, NT * 3], dt.bfloat16, tag="FL")
            FLR = FL[:].rearrange("p (n c) -> p n c", c=3)
            M2 = bpool.tile([128, NT], dt.bfloat16, tag="M2")
            TMX = bpool.tile([128, NT], dt.bfloat16, tag="TMX")

            # comps: [a, b, alo, blo, am, bm, amlo, bmlo,
            #         opA, opS, opAm, opSm, 1, mA, 0, 0]
            CB = bpool.tile([128, NT * 32], dt.bfloat16, tag="CB")
            CB4 = CB[:].rearrange("p (n s c) -> p n s c", s=2, c=16)
            # xbar group view: [p, grp, (4 tiles * 2 pos * 16 comp)=128]
            CBG = CB[:].rearrange("p (g c) -> p g c", c=128)

            CT = bpool.tile([128, (NT // 4) * 128], dt.bfloat16, tag="CT")
            CTR = CT[:].rearrange("p (g t) -> p g t", t=128)

            # RH: [p=hid, (slot(ph*4+q), k(st*2+half), gg(4), tok(128))] fp16
            RH = bpool.tile([128, 8 * 4 * 4 * 128], dt.float16, tag="RH")
            RHv = RH[:].rearrange("p (s k g t) -> p s k g t", k=4, g=4, t=128)

            RESS = bpool.tile([128, NT * 4], dt.float32, tag="RESS")
            RESV = RESS[:].rearrange("p (n s w) -> p n s w", s=2, w=2)
            RD = bpool.tile([128, NT * 2], dt.float32, tag="RD")
            RDV = RD[:].rearrange("p (n s) -> p n s", s=2)
            RSEL = bpool.tile([128, NT * 2], dt.float32, tag="RSEL")
            RSV = RSEL[:].rearrange("p (n s) -> p n s", s=2)
            RSB = bpool.tile([128, NT * 2], dt.bfloat16, tag="RSB")
            RSBV = RSB[:].rearrange("p (n s) -> p n s", s=2)
            EQ = bpool.tile([128, NT * 32], dt.bfloat16, tag="EQ")
            EQ4 = EQ[:].rearrange("p (n s k) -> p n s k", s=2, k=16)

            # CB static init: zeros + ones at comp 12
            nc.gpsimd.memset(CB[:], 0.0)
            nc.gpsimd.memset(CB4[:, :, :, 12:13], 1.0)

            # ---- input DMA chunks (SP/HWDGE ring) ----
            for c in range(NCHUNK):
                t0 = c * tpc
                nc.sync.dma_start(XR[:, t0:t0 + tpc, :], xr[:, t0:t0 + tpc, :])

            def decode_cbuild(st):
                n0 = st * G
                sl = slice(n0, n0 + G)
                # tsel = (x > 0.5) * (k - 16), bf16
                nc.vector.scalar_tensor_tensor(
                    out=HR[:, sl, :],
                    in0=XR[:, sl, WIN0:WIN0 + 64],
                    scalar=0.5,
                    in1=K16[:, None, :].to_broadcast([128, G, 64]),
                    op0=Alu.is_gt, op1=Alu.mult)
                # per-16 min reduce -> NIB (first-hit-16; 0 if no hit)
                red_eng = nc.gpsimd if REDUCE_ON_GPSIMD else nc.vector
                red_eng.tensor_reduce(
                    out=NIB4[:, sl, :],
                    in_=HR[:, sl, :].rearrange("p n (w k) -> p n w k", k=16),
                    axis=mybir.AxisListType.X, op=Alu.min)
                nc.vector.tensor_scalar(
                    out=NM4[:, sl, :], in0=NIB4[:, sl, :],
                    scalar1=-0.5, scalar2=None, op0=Alu.is_lt)
                # flags
                nc.vector.tensor_scalar(
                    out=FLR[:, sl, :], in0=XR[:, sl, 0:3],
                    scalar1=0.5, scalar2=None, op0=Alu.is_gt)
                nc.vector.tensor_tensor(
                    out=TMX[:, sl], in0=FLR[:, sl, 1], in1=FLR[:, sl, 2],
                    op=Alu.max)
                nc.vector.scalar_tensor_tensor(
                    out=M2[:, sl], in0=TMX[:, sl], scalar=2.0,
                    in1=FLR[:, sl, 0], op0=Alu.mult, op1=Alu.mult)

                # ---- c build ----
                CBst = CB4[:, sl, :, :]
                # cols 0:4 = [a, b] x {hi, lo-dup} = (nib+16)*nm, per pos
                for s in range(2):
                    # windows w = (ab, pos): a at w=s, b at w=2+s
                    nib_s = NIB4[:, sl, s:s + 3:2]
                    nm_s = NM4[:, sl, s:s + 3:2]
                    for d in range(2):
                        nc.vector.scalar_tensor_tensor(
                            out=CBst[:, :, s, 2 * d:2 * d + 2],
                            in0=nib_s, scalar=16.0, in1=nm_s,
                            op0=Alu.add, op1=Alu.mult)
                # cols 4:8 = cols 0:4 * mA
                nc.vector.tensor_tensor(
                    out=CBst[:, :, :, 4:8], in0=CBst[:, :, :, 0:4],
                    in1=FLR[:, sl, None, None, 1].to_broadcast([128, G, 2, 4]),
                    op=Alu.mult)
                # cols 8:10 = opA, opS
                for s in range(2):
                    nc.vector.tensor_copy(
                        CBst[:, :, s, 8:10], XR[:, sl, OPA:OPS + 1])
                # cols 10:12 = (opA, opS) * mA
                nc.vector.tensor_tensor(
                    out=CBst[:, :, :, 10:12], in0=CBst[:, :, :, 8:10],
                    in1=FLR[:, sl, None, None, 1].to_broadcast([128, G, 2, 2]),
                    op=Alu.mult)
                # col 13 = mA
                nc.vector.tensor_copy(
                    CBst[:, :, :, 13:14],
                    FLR[:, sl, None, None, 1].to_broadcast([128, G, 2, 1]))

            def transposes(st):
                for j in range(8):
                    g = st * 8 + j
                    nc.sync.dma_start_transpose(CTR[:, g, :], CBG[:, g, :])

            def h_matmuls(st):
                for ph in range(2):
                    for q in range(4):
                        hp = hp_pool.tile([128, 1024], dt.float32, tag="hp")
                        for half in range(2):
                            g0 = st * 8 + half * 4
                            rhs = CTR[32 * q:32 * q + 32, g0:g0 + 4, :]
                            nc.tensor.matmul(
                                hp[:, half * 512:(half + 1) * 512],
                                W16[32 * q:32 * q + 32,
                                    128 * ph:128 * ph + 128],
                                rhs,
                                start=True, stop=True,
                                tile_position=(32 * q, 0))
                        # relu -> RH slot (ph*4+q), k = 2*st..2*st+2
                        out = RHv[:, ph * 4 + q, 2 * st:2 * st + 2, :, :]
                        nc.scalar.activation(out, hp[:], Act.Relu)

            def layer2(st):
                rp = rp_pool.tile([128, 128], dt.float32, tag="rp")
                for nn in range(G):
                    n = st * G + nn
                    q, gg, k2 = n % 4, (n // 4) % 4, n // 16
                    for ph in range(2):
                        lhsT = RHv[:, ph * 4 + q, k2, gg, :]
                        nc.tensor.matmul(
                            rp[:, 4 * nn + 2 * ph:4 * nn + 2 * ph + 2],
                            lhsT, W2[:], start=True, stop=True)
                return rp

            def post(st, rp):
                n0 = st * G
                sl = slice(n0, n0 + G)
                rpv = rp[:].rearrange("p (n s w) -> p n s w", s=2, w=2)
                nc.vector.tensor_copy(RESV[:, sl, :, :], rpv)
                # rsel = res_sub + mA*(res_add - res_sub); w0=add, w1=sub
                nc.vector.tensor_tensor(out=RDV[:, sl, :],
                                        in0=RESV[:, sl, :, 0],
                                        in1=RESV[:, sl, :, 1], op=Alu.subtract)
                nc.vector.tensor_tensor(
                    out=RDV[:, sl, :], in0=RDV[:, sl, :],
                    in1=FLR[:, sl, None, 1].to_broadcast([128, G, 2]),
                    op=Alu.mult)
                nc.vector.tensor_tensor(out=RSV[:, sl, :], in0=RDV[:, sl, :],
                                        in1=RESV[:, sl, :, 1], op=Alu.add)
                # round (RNE), +100, clamp to [100,115]
                nc.vector.tensor_scalar(out=RSEL[:, 2 * n0:2 * n0 + 2 * G],
                                        in0=RSEL[:, 2 * n0:2 * n0 + 2 * G],
                                        scalar1=ROUND_C,
                                        scalar2=ROUND_C - 100.0,
                                        op0=Alu.add, op1=Alu.subtract)
                nc.vector.tensor_scalar(out=RSEL[:, 2 * n0:2 * n0 + 2 * G],
                                        in0=RSEL[:, 2 * n0:2 * n0 + 2 * G],
                                        scalar1=100.0, scalar2=115.0,
                                        op0=Alu.max, op1=Alu.min)
                # fold processed mask: r' = r+100-50*m2 (in [0,15] iff m2=2)
                nc.vector.scalar_tensor_tensor(
                    out=RSBV[:, sl, :],
                    in0=M2[:, sl, None].to_broadcast([128, G, 2]),
                    scalar=-50.0, in1=RSV[:, sl, :], op0=Alu.mult, op1=Alu.add)
                nc.vector.tensor_tensor(
                    out=EQ4[:, sl, :, :],
                    in0=IOTA[:].rearrange("p (s k) -> p s k", s=2)[:, None]
                        .to_broadcast([128, G, 2, 16]),
                    in1=RSBV[:, sl, :, None].to_broadcast([128, G, 2, 16]),
                    op=Alu.is_equal)
                nc.vector.scalar_tensor_tensor(
                    out=XR[:, sl, OUT_LO:OUT_LO + 32],
                    in0=EQ[:].rearrange("p (n c) -> p n c", c=32)[:, sl, :],
                    scalar=2.0,
                    in1=XR[:, sl, OUT_LO:OUT_LO + 32],
                    op0=Alu.mult, op1=Alu.add)

            # ---------- schedule ----------
            rps = [None] * NST
            for st in range(NST):
                decode_cbuild(st)
                transposes(st)
                h_matmuls(st)
                rps[st] = layer2(st)
            for st in range(NST):
                post(st, rps[st])

            # ---- output DMA (gpsimd SWDGE ring) ----
            for c in range(NCHUNK):
                t0 = c * tpc
                nc.gpsimd.dma_start(yr[:, t0:t0 + tpc, :], XR[:, t0:t0 + tpc, :])

    nc.compile()
    return nc


def make_consts(W_add1, b_add1, W_add2, b_add2, W_sub1, b_sub1, W_sub2, b_sub2):
    f32 = np.float32
    bf16 = ml_dtypes.bfloat16
    rows = [0, 1, 27, 28]     # GE comps: NIB_A, NIB_B, OP_START+25, OP_START+26

    def eff(W1, b1):
        return np.concatenate([np.asarray(W1, f32)[rows, :],
                               np.asarray(b1, f32)[None, :]], axis=0)

    es = eff(W_sub1, b_sub1)
    ea = eff(W_add1, b_add1)
    blk = np.zeros((10, 128), f32)
    blk[0:5] = es
    blk[5:10] = (ea.astype(np.float64) - es.astype(np.float64)).astype(f32)
    bhi = blk.astype(bf16)
    blo = (blk - bhi.astype(f32)).astype(bf16)
    # comp order: [a,b, alo,blo, am,bm, amlo,bmlo, opA,opS, opAm,opSm, 1, mA]
    blk14 = np.stack([bhi[0], bhi[1], blo[0], blo[1],
                      bhi[5], bhi[6], blo[5], blo[6],
                      bhi[2], bhi[3], bhi[7], bhi[8],
                      bhi[4], bhi[9]], axis=0)  # [14, 128]

    w16 = np.zeros((128, 256), bf16)
    for q in range(4):
        w16[32 * q:32 * q + 14, 0:128] = blk14        # phase A (pos 0)
        w16[32 * q + 16:32 * q + 30, 128:256] = blk14  # phase B (pos 1)

    w2 = np.stack([np.asarray(W_add2, f32)[:, GE_RESULT],
                   np.asarray(W_sub2, f32)[:, GE_RESULT]],
                  axis=1).astype(np.float16)

    iota = np.broadcast_to(np.tile(np.arange(16, dtype=f32), 2),
                           (128, 32)).astype(bf16).copy()
    k16 = np.broadcast_to((np.arange(64, dtype=f32) % 16) - 16.0,
                          (128, 64)).astype(bf16).copy()
    return dict(cW16AB=w16, cW2=w2, cIOTA=iota, cK16=k16)


_NC_CACHE = {}


def _get_nc():
    if "nc" not in _NC_CACHE:
        _NC_CACHE["nc"] = build_nc()
    return _NC_CACHE["nc"]


def kernel(x_bd, W_add1, b_add1, W_add2, b_add2, W_sub1, b_sub1, W_sub2, b_sub2):
    from concourse import bass_utils

    x = np.ascontiguousarray(np.asarray(x_bd, dtype=np.float32)).reshape(TOK, D)
    consts = make_consts(W_add1, b_add1, W_add2, b_add2,
                         W_sub1, b_sub1, W_sub2, b_sub2)
    badd2 = float(np.asarray(b_add2)[GE_RESULT])
    bsub2 = float(np.asarray(b_sub2)[GE_RESULT])
    assert badd2 == 0.0 and bsub2 == 0.0, "nonzero output bias not folded"

    nc = _get_nc()
    in_maps = []
    for c in range(NCORES):
        m = dict(consts)
        m["xc"] = x[c * TPC:(c + 1) * TPC]
        in_maps.append(m)
    res = bass_utils.run_bass_kernel_spmd(nc, in_maps, list(range(NCORES)))
    y = np.concatenate([res.results[c]["yc"] for c in range(NCORES)], axis=0)
    return y.reshape(B, S, D)


if __name__ == "__main__":
    build_nc()
    print("built ok")


# revision 15
# speedup vs baseline: 1.1968x; 1.1968x over previous
"""Trainium2 Bass kernel for nn_Efficient8BitALU_AddSub (v2).

Contract: kernel(**inputs) takes FULL unsharded inputs (numpy), returns FULL
output [32, 2048, 128] float32.  Internally shards tokens across 8 NeuronCores
(pure data parallel), runs a Bass/Tile kernel per core, gathers.

v2 redesign vs baseline (63.6us):
  - DMA layout "(p n) f": each partition holds 64 consecutive tokens, so every
    HBM descriptor moves 4KB contiguous (was 512B rows) -> near line rate.
  - c-vector transpose moved off PE onto the DMA xbar (dma_start_transpose,
    SBUF->SBUF, doesn't touch HBM), killing 64 PE transposes + ACT copies.
  - h-matmul: 32 N=512 matmuls with masked A/B weight blocks per 32-row band
    (pos0 comps at band rows 0:16, pos1 at 16:32), W16 replicated at the four
    row-group offsets.  layer2 keeps lhsT=RH chunks (tokens end on partitions).
  - decode in bf16, window min-reduce on gpsimd, fewer/bigger DVE ops.
  - outputs DMA'd via gpsimd SWDGE ring so the SP ring only carries inputs +
    xbar transposes (no FIFO stalls).
"""

import sys

import numpy as np

sys.path.insert(0, "/opt/trn_rl_repo")

import ml_dtypes  # noqa: E402
import concourse.bacc as bacc  # noqa: E402
import concourse.bass as bass  # noqa: E402
import concourse.mybir as mybir  # noqa: E402
import concourse.tile as tile  # noqa: E402

dt = mybir.dt
Alu = mybir.AluOpType
Act = mybir.ActivationFunctionType

# ---- problem constants (hardcoded per contract) ----
B, S, D = 32, 2048, 128
NCORES = 8
TOK = B * S                   # 65536
TPC = TOK // NCORES           # 8192 tokens per core

WIN0 = 3                      # 4 contiguous 16-wide decode windows: 3..66
OUT_LO = 67                   # outputs 67..98 (lo 67:83, hi 83:99)
OPA, OPS = 124, 125
GE_RESULT = 63
ROUND_C = 12582912.0          # 1.5 * 2**23 : RNE round-to-integer magic

NT = TPC // 128               # 64 tiles; token t = p*NT + n
NST = 2                       # supertiles
G = NT // NST                 # 32 tiles per supertile
NCHUNK = 8                    # dma chunks of 8 tiles

REDUCE_ON_GPSIMD = False  # gpsimd tensor_reduce is partition-axis only


def build_nc():
    nc = bacc.Bacc("TRN2", target_bir_lowering=False, debug=False,
                   num_devices=NCORES)
    xd = nc.dram_tensor("xc", [TPC, D], dt.float32, kind="ExternalInput")
    w16d = nc.dram_tensor("cW16AB", [128, 256], dt.bfloat16, kind="ExternalInput")
    w2d = nc.dram_tensor("cW2", [128, 2], dt.float16, kind="ExternalInput")
    iotad = nc.dram_tensor("cIOTA", [128, 32], dt.bfloat16, kind="ExternalInput")
    k16d = nc.dram_tensor("cK16", [128, 64], dt.bfloat16, kind="ExternalInput")
    yd = nc.dram_tensor("yc", [TPC, D], dt.float32, kind="ExternalOutput")

    # token t = p*NT + n : per-partition-contiguous DRAM blocks
    xr = xd.ap().rearrange("(p n) f -> p n f", n=NT)
    yr = yd.ap().rearrange("(p n) f -> p n f", n=NT)

    tpc = NT // NCHUNK        # tiles per dma chunk = 8

    with tile.TileContext(nc) as tc:
        with (
            tc.tile_pool(name="const", bufs=1) as cpool,
            tc.tile_pool(name="big", bufs=1) as bpool,
            tc.tile_pool(name="hp", bufs=3, space="PSUM") as hp_pool,
            tc.tile_pool(name="rp", bufs=2, space="PSUM") as rp_pool,
        ):
            W16 = cpool.tile([128, 256], dt.bfloat16, tag="w16")
            W2 = cpool.tile([128, 2], dt.float16, tag="w2")
            IOTA = cpool.tile([128, 32], dt.bfloat16, tag="iota")
            K16 = cpool.tile([128, 64], dt.bfloat16, tag="k16")
            nc.gpsimd.dma_start(W16[:], w16d.ap())
            nc.gpsimd.dma_start(W2[:], w2d.ap())
            nc.gpsimd.dma_start(IOTA[:], iotad.ap())
            nc.gpsimd.dma_start(K16[:], k16d.ap())

            X = bpool.tile([128, NT * 128], dt.float32, tag="X")
            XR = X[:].rearrange("p (n f) -> p n f", f=128)

            HT = bpool.tile([128, NT * 64], dt.bfloat16, tag="HT")
            HR = HT[:].rearrange("p (n w) -> p n w", w=64)
            NIB = bpool.tile([128, NT * 4], dt.bfloat16, tag="NIB")
            NIB4 = NIB[:].rearrange("p (n w) -> p n w", w=4)
            NM = bpool.tile([128, NT * 4], dt.bfloat16, tag="NM")
            NM4 = NM[:].rearrange("p (n w) -> p n w", w=4)
            FL = bpool.tile([128, NT * 3], dt.bfloat16, tag="FL")
            FLR = FL[:].rearrange("p (n c) -> p n c", c=3)
            M2 = bpool.tile([128, NT], dt.bfloat16, tag="M2")
            TMX = bpool.tile([128, NT], dt.bfloat16, tag="TMX")

            # comps: [a, b, alo, blo, am, bm, amlo, bmlo,
            #         opA, opS, opAm, opSm, 1, mA, 0, 0]
            CB = bpool.tile([128, NT * 32], dt.bfloat16, tag="CB")
            CB4 = CB[:].rearrange("p (n s c) -> p n s c", s=2, c=16)
            # xbar group view: [p, grp, (4 tiles * 2 pos * 16 comp)=128]
            CBG = CB[:].rearrange("p (g c) -> p g c", c=128)

            CT = bpool.tile([128, (NT // 4) * 128], dt.bfloat16, tag="CT")
            CTR = CT[:].rearrange("p (g t) -> p g t", t=128)

            # RH: [p=hid, (slot(ph*4+q), k(st*2+half), gg(4), tok(128))] fp16
            RH = bpool.tile([128, 8 * 4 * 4 * 128], dt.float16, tag="RH")
            RHv = RH[:].rearrange("p (s k g t) -> p s k g t", k=4, g=4, t=128)

            RESS = bpool.tile([128, NT * 4], dt.float32, tag="RESS")
            RESV = RESS[:].rearrange("p (n s w) -> p n s w", s=2, w=2)
            RD = bpool.tile([128, NT * 2], dt.float32, tag="RD")
            RDV = RD[:].rearrange("p (n s) -> p n s", s=2)
            RSEL = bpool.tile([128, NT * 2], dt.float32, tag="RSEL")
            RSV = RSEL[:].rearrange("p (n s) -> p n s", s=2)
            RSB = bpool.tile([128, NT * 2], dt.bfloat16, tag="RSB")
            RSBV = RSB[:].rearrange("p (n s) -> p n s", s=2)
            EQ = bpool.tile([128, NT * 32], dt.bfloat16, tag="EQ")
            EQ4 = EQ[:].rearrange("p (n s k) -> p n s k", s=2, k=16)

            # CB static init: zeros + ones at comp 12
            nc.gpsimd.memset(CB[:], 0.0)
            nc.gpsimd.memset(CB4[:, :, :, 12:13], 1.0)

            # ---- input DMA chunks (SP/HWDGE ring) ----
            for c in range(NCHUNK):
                t0 = c * tpc
                nc.sync.dma_start(XR[:, t0:t0 + tpc, :], xr[:, t0:t0 + tpc, :])

            def decode_chunk(c):
                n0 = c * tpc
                sl = slice(n0, n0 + tpc)
                # tsel = (x > 0.5) * (k - 16), bf16
                nc.vector.scalar_tensor_tensor(
                    out=HR[:, sl, :],
                    in0=XR[:, sl, WIN0:WIN0 + 64],
                    scalar=0.5,
                    in1=K16[:, None, :].to_broadcast([128, tpc, 64]),
                    op0=Alu.is_gt, op1=Alu.mult)
                # per-16 min reduce -> NIB (first-hit-16; 0 if no hit)
                nc.vector.tensor_reduce(
                    out=NIB4[:, sl, :],
                    in_=HR[:, sl, :].rearrange("p n (w k) -> p n w k", k=16),
                    axis=mybir.AxisListType.X, op=Alu.min)

            def decode_cbuild(st):
                n0 = st * G
                sl = slice(n0, n0 + G)
                nc.vector.tensor_scalar(
                    out=NM4[:, sl, :], in0=NIB4[:, sl, :],
                    scalar1=-0.5, scalar2=None, op0=Alu.is_lt)
                # flags
                nc.vector.tensor_scalar(
                    out=FLR[:, sl, :], in0=XR[:, sl, 0:3],
                    scalar1=0.5, scalar2=None, op0=Alu.is_gt)
                nc.vector.tensor_tensor(
                    out=TMX[:, sl], in0=FLR[:, sl, 1], in1=FLR[:, sl, 2],
                    op=Alu.max)
                nc.vector.scalar_tensor_tensor(
                    out=M2[:, sl], in0=TMX[:, sl], scalar=2.0,
                    in1=FLR[:, sl, 0], op0=Alu.mult, op1=Alu.mult)

                # ---- c build ----
                CBst = CB4[:, sl, :, :]
                # cols 0:4 = [a, b] x {hi, lo-dup} = (nib+16)*nm, per pos
                for s in range(2):
                    # windows w = (ab, pos): a at w=s, b at w=2+s
                    nib_s = NIB4[:, sl, s:s + 3:2]
                    nm_s = NM4[:, sl, s:s + 3:2]
                    for d in range(2):
                        nc.vector.scalar_tensor_tensor(
                            out=CBst[:, :, s, 2 * d:2 * d + 2],
                            in0=nib_s, scalar=16.0, in1=nm_s,
                            op0=Alu.add, op1=Alu.mult)
                # cols 4:8 = cols 0:4 * mA
                nc.vector.tensor_tensor(
                    out=CBst[:, :, :, 4:8], in0=CBst[:, :, :, 0:4],
                    in1=FLR[:, sl, None, None, 1].to_broadcast([128, G, 2, 4]),
                    op=Alu.mult)
                # cols 8:10 = opA, opS
                for s in range(2):
                    nc.vector.tensor_copy(
                        CBst[:, :, s, 8:10], XR[:, sl, OPA:OPS + 1])
                # cols 10:12 = (opA, opS) * mA
                nc.vector.tensor_tensor(
                    out=CBst[:, :, :, 10:12], in0=CBst[:, :, :, 8:10],
                    in1=FLR[:, sl, None, None, 1].to_broadcast([128, G, 2, 2]),
                    op=Alu.mult)
                # col 13 = mA
                nc.vector.tensor_copy(
                    CBst[:, :, :, 13:14],
                    FLR[:, sl, None, None, 1].to_broadcast([128, G, 2, 1]))

            def transposes(st):
                g0 = st * 8
                nc.sync.dma_start_transpose(
                    CTR[:, g0:g0 + 8, :], CBG[:, g0:g0 + 8, :].rearrange(
                        "p g c -> p (g c)"))

            def h_matmuls(st):
                for ph in range(2):
                    for q in range(4):
                        hp = hp_pool.tile([128, 1024], dt.float32, tag="hp")
                        for half in range(2):
                            g0 = st * 8 + half * 4
                            rhs = CTR[32 * q:32 * q + 32, g0:g0 + 4, :]
                            nc.tensor.matmul(
                                hp[:, half * 512:(half + 1) * 512],
                                W16[32 * q:32 * q + 32,
                                    128 * ph:128 * ph + 128],
                                rhs,
                                start=True, stop=True,
                                tile_position=(32 * q, 0))
                        # relu -> RH slot (ph*4+q), k = 2*st..2*st+2
                        out = RHv[:, ph * 4 + q, 2 * st:2 * st + 2, :, :]
                        nc.scalar.activation(out, hp[:], Act.Relu)

            def layer2(st):
                # emit in relu-completion order (ph, q major)
                rp = rp_pool.tile([128, 128], dt.float32, tag="rp")
                for ph in range(2):
                    for q in range(4):
                        for h2 in range(2):
                            for gg in range(4):
                                nn = 16 * h2 + 4 * gg + q
                                k2 = 2 * st + h2
                                lhsT = RHv[:, ph * 4 + q, k2, gg, :]
                                nc.tensor.matmul(
                                    rp[:, 4 * nn + 2 * ph:4 * nn + 2 * ph + 2],
                                    lhsT, W2[:], start=True, stop=True)
                return rp

            def post(st, h2, rp):
                GH = G // 2
                n0 = st * G + h2 * GH
                sl = slice(n0, n0 + GH)
                rpv = rp[:].rearrange("p (n s w) -> p n s w", s=2, w=2)[
                    :, h2 * GH:(h2 + 1) * GH, :, :]
                nc.vector.tensor_copy(RESV[:, sl, :, :], rpv)
                # rsel = res_sub + mA*(res_add - res_sub); w0=add, w1=sub
                nc.vector.tensor_tensor(out=RDV[:, sl, :],
                                        in0=RESV[:, sl, :, 0],
                                        in1=RESV[:, sl, :, 1], op=Alu.subtract)
                nc.vector.tensor_tensor(
                    out=RDV[:, sl, :], in0=RDV[:, sl, :],
                    in1=FLR[:, sl, None, 1].to_broadcast([128, GH, 2]),
                    op=Alu.mult)
                nc.vector.tensor_tensor(out=RSV[:, sl, :], in0=RDV[:, sl, :],
                                        in1=RESV[:, sl, :, 1], op=Alu.add)
                # round (RNE), +100, clamp to [100,115]
                nc.vector.tensor_scalar(out=RSEL[:, 2 * n0:2 * n0 + 2 * GH],
                                        in0=RSEL[:, 2 * n0:2 * n0 + 2 * GH],
                                        scalar1=ROUND_C,
                                        scalar2=ROUND_C - 100.0,
                                        op0=Alu.add, op1=Alu.subtract)
                nc.vector.tensor_scalar(out=RSEL[:, 2 * n0:2 * n0 + 2 * GH],
                                        in0=RSEL[:, 2 * n0:2 * n0 + 2 * GH],
                                        scalar1=100.0, scalar2=115.0,
                                        op0=Alu.max, op1=Alu.min)
                # fold processed mask: r' = r+100-50*m2 (in [0,15] iff m2=2)
                nc.vector.scalar_tensor_tensor(
                    out=RSBV[:, sl, :],
                    in0=M2[:, sl, None].to_broadcast([128, GH, 2]),
                    scalar=-50.0, in1=RSV[:, sl, :], op0=Alu.mult, op1=Alu.add)
                nc.vector.tensor_tensor(
                    out=EQ4[:, sl, :, :],
                    in0=IOTA[:].rearrange("p (s k) -> p s k", s=2)[:, None]
                        .to_broadcast([128, GH, 2, 16]),
                    in1=RSBV[:, sl, :, None].to_broadcast([128, GH, 2, 16]),
                    op=Alu.is_equal)
                nc.vector.scalar_tensor_tensor(
                    out=XR[:, sl, OUT_LO:OUT_LO + 32],
                    in0=EQ[:].rearrange("p (n c) -> p n c", c=32)[:, sl, :],
                    scalar=2.0,
                    in1=XR[:, sl, OUT_LO:OUT_LO + 32],
                    op0=Alu.mult, op1=Alu.add)

            # ---------- schedule ----------
            cpst = NCHUNK // NST          # dma chunks per supertile
            rps = [None] * NST

            def out_chunk(c):
                t0 = c * tpc
                nc.gpsimd.dma_start(yr[:, t0:t0 + tpc, :],
                                    XR[:, t0:t0 + tpc, :])

            for st in range(NST):
                for c in range(st * cpst, (st + 1) * cpst):
                    decode_chunk(c)
                decode_cbuild(st)
                transposes(st)
                h_matmuls(st)
                rps[st] = layer2(st)
            for st in range(NST):
                for h2 in range(2):
                    post(st, h2, rps[st])
                    c0 = st * cpst + h2 * (cpst // 2)
                    for c in range(c0, c0 + cpst // 2):
                        out_chunk(c)

    nc.compile()
    return nc


def make_consts(W_add1, b_add1, W_add2, b_add2, W_sub1, b_sub1, W_sub2, b_sub2):
    f32 = np.float32
    bf16 = ml_dtypes.bfloat16
    rows = [0, 1, 27, 28]     # GE comps: NIB_A, NIB_B, OP_START+25, OP_START+26

    def eff(W1, b1):
        return np.concatenate([np.asarray(W1, f32)[rows, :],
                               np.asarray(b1, f32)[None, :]], axis=0)

    es = eff(W_sub1, b_sub1)
    ea = eff(W_add1, b_add1)
    blk = np.zeros((10, 128), f32)
    blk[0:5] = es
    blk[5:10] = (ea.astype(np.float64) - es.astype(np.float64)).astype(f32)
    bhi = blk.astype(bf16)
    blo = (blk - bhi.astype(f32)).astype(bf16)
    # comp order: [a,b, alo,blo, am,bm, amlo,bmlo, opA,opS, opAm,opSm, 1, mA]
    blk14 = np.stack([bhi[0], bhi[1], blo[0], blo[1],
                      bhi[5], bhi[6], blo[5], blo[6],
                      bhi[2], bhi[3], bhi[7], bhi[8],
                      bhi[4], bhi[9]], axis=0)  # [14, 128]

    w16 = np.zeros((128, 256), bf16)
    for q in range(4):
        w16[32 * q:32 * q + 14, 0:128] = blk14        # phase A (pos 0)
        w16[32 * q + 16:32 * q + 30, 128:256] = blk14  # phase B (pos 1)

    w2 = np.stack([np.asarray(W_add2, f32)[:, GE_RESULT],
                   np.asarray(W_sub2, f32)[:, GE_RESULT]],
                  axis=1).astype(np.float16)

    iota = np.broadcast_to(np.tile(np.arange(16, dtype=f32), 2),
                           (128, 32)).astype(bf16).copy()
    k16 = np.broadcast_to((np.arange(64, dtype=f32) % 16) - 16.0,
                          (128, 64)).astype(bf16).copy()
    return dict(cW16AB=w16, cW2=w2, cIOTA=iota, cK16=k16)


_NC_CACHE = {}


def _get_nc():
    if "nc" not in _NC_CACHE:
        _NC_CACHE["nc"] = build_nc()
    return _NC_CACHE["nc"]


def kernel(x_bd, W_add1, b_add1, W_add2, b_add2, W_sub1, b_sub1, W_sub2, b_sub2):
    from concourse import bass_utils

    x = np.ascontiguousarray(np.asarray(x_bd, dtype=np.float32)).reshape(TOK, D)
    consts = make_consts(W_add1, b_add1, W_add2, b_add2,
                         W_sub1, b_sub1, W_sub2, b_sub2)
    badd2 = float(np.asarray(b_add2)[GE_RESULT])
    bsub2 = float(np.asarray(b_sub2)[GE_RESULT])
    assert badd2 == 0.0 and bsub2 == 0.0, "nonzero output bias not folded"

    nc = _get_nc()
    in_maps = []
    for c in range(NCORES):
        m = dict(consts)
        m["xc"] = x[c * TPC:(c + 1) * TPC]
        in_maps.append(m)
    res = bass_utils.run_bass_kernel_spmd(nc, in_maps, list(range(NCORES)))
    y = np.concatenate([res.results[c]["yc"] for c in range(NCORES)], axis=0)
    return y.reshape(B, S, D)


if __name__ == "__main__":
    build_nc()
    print("built ok")
